# revision 21
# baseline (speedup 1.0000x reference)
"""ColWarp (per-sample color warp + shift + depthwise 5x5 conv) on 8 TRN2 cores.

Decomposition: out[c] = conv5x5(sum_d W[d,c]*(im[d]+shift[d]), k) is linear,
so the 3x3 color warp, the per-channel shift, and the 5-tap column conv all
fold into per-sample banded Toeplitz stationary matrices built on host from
flat_col (32x37).  Each output row-tile is 5 PSUM-accumulated fp16 matmul
pairs (one per kernel column dx, the dx shift expressed as a free-dim offset
into a zero-padded rhs tile).  The shift term rides on a constant ones row
whose stationary coefficient is s'[c] * (sum of dy-valid kernel taps), which
reproduces the zero-padding border behavior exactly.

v2 schedule (PE is the bottleneck at ~131us busy; everything else must hide
behind it):
  - 20 full 38-row h-tiles per sample; the 8-row remainder strip of two
    samples is packed into one 73-contraction stationary (48 psum rows), so
    the remainder costs 2 pass-sets instead of 4.
  - dx-outer matmul order with two 384-wide psum chunks per tile: every
    LDWEIGHTS (~118ns) hides behind a 384-col matmul (~160ns), instead of
    spilling ~8ns behind each 256-col matmul.
  - lean head: first input tile + first 5 stationaries are DMA'd first and
    a short warmup ramp (NWARM matmuls) covers exactly the DMA wait.
  - psum->sbuf copies split across vector AND scalar engines per tile.
  - output pieces stream held-by-one; the final pieces shrink so the drain
    after the last matmul is small.

Data parallel: 4 samples per core, 8 cores, no cross-core communication.
"""

import numpy as np

import concourse.bass as bass
import concourse.mybir as mybir
import concourse.tile as tile
from concourse.bass_utils import run_bass_kernel_spmd

BS, C, H, W = 32, 3, 768, 768
NCORES = 8
SPC = BS // NCORES  # 4 samples per core

ROWS = 38  # output rows per full h-tile
WIN = ROWS + 4  # input window rows per channel (42)
NT = 20  # full h-tiles (rows 0..760); remainder strip handled separately
KC = 3 * WIN + 1  # matmul contraction: 126 image rows + ones row
ONES = 3 * WIN  # partition index of the ones row (126)
FREE = W + 4  # rhs width incl. 2+2 zero side cols (772)
NCLS = 2  # stationary classes: first / interior tile
MT_FULL = 3 * ROWS  # 114
OPC = 7  # max h-tiles per output DMA piece

# remainder strip: rows 760..768, two samples packed per pass-set
REM_ROWS = H - NT * ROWS  # 8
REM_WIN = REM_ROWS + 4  # 12
REM_KC = 2 * 3 * REM_WIN + 1  # 73 (two samples' windows + ones row)
REM_ONES = 2 * 3 * REM_WIN  # 72
REM_MT = 2 * 3 * REM_ROWS  # 48
REM_A = NT * ROWS  # 760
NPAIR = SPC // 2  # 2

NSTAT = NCLS * 5  # main stationaries per sample (10)
NBLK = SPC * NSTAT + NPAIR * 5  # 50 stationary blocks total
STATW = NBLK * 128

# input DMA pieces (tile ranges) per sample: sample 0 starts fine-grained so
# the first matmul's data lands early; the rest use full-width pieces.
IN_PIECES_B0 = [(0, 1), (1, 2), (2, 4), (4, 7), (7, 10), (10, 14), (14, 17), (17, 20)]
IN_PIECES = [(0, 7), (7, 14), (14, 20)]
OUT_PIECES = [(0, 7), (7, 14), (14, 20)]
# last sample's pieces shrink so the end-of-kernel drain is tiny.
OUT_PIECES_LAST = [(0, 7), (7, 14), (14, 17), (17, 19), (19, 20)]

_nc_cache = {}


def _legalize_waits(nc):
    # This walrus build rejects >1 sync wait per instruction; move extra
    # waits onto same-engine NOPs immediately before (sequencers execute
    # waits in program order, so this is equivalent).
    for f in nc.m.functions:
        for blk in f.blocks:
            out = []
            changed = False
            for inst in blk.instructions:
                si = inst.sync_info
                waits = list(si.on_wait) if si is not None and si.on_wait else []
                if len(waits) > 1:
                    changed = True
                    for j, w in enumerate(waits[:-1]):
                        out.append(
                            mybir.InstNoOp(
                                name=f"{inst.name}-wsplit{j}",
                                engine=inst.engine,
                                ins=[],
                                outs=[],
                                sync_info=mybir.SyncInfo(on_wait=[w], on_update=[]),
                            )
                        )
                    inst.sync_info = mybir.SyncInfo(
                        on_wait=[waits[-1]],
                        on_update=list(si.on_update) if si.on_update else [],
                    )
                out.append(inst)
            if changed:
                blk.instructions = out


def _build_program():
    f32 = mybir.dt.float32
    f16 = mybir.dt.float16
    nc = bass.Bass()
    im_dram = nc.declare_dram_parameter(
        "im_tiled", [SPC, KC, NT, FREE], f16, isOutput=False
    )
    rem_dram = nc.declare_dram_parameter(
        "im_rem", [NPAIR, REM_KC, FREE], f16, isOutput=False
    )
    stat_dram = nc.declare_dram_parameter("stat", [128, STATW], f16, isOutput=False)
    out_dram = nc.declare_dram_parameter(
        "out_tiled", [SPC, MT_FULL, NT, W], f16, isOutput=True
    )
    outr_dram = nc.declare_dram_parameter(
        "out_rem", [NPAIR, REM_MT, W], f16, isOutput=True
    )

    with tile.TileContext(nc) as tc:
        with (
            tc.tile_pool(name="stat", bufs=1) as stat_pool,
            tc.tile_pool(name="rem", bufs=1) as rem_pool,
            tc.tile_pool(name="warm", bufs=1) as warm_pool,
            tc.tile_pool(name="rhs", bufs=3) as rhs_pool,
            tc.tile_pool(name="outb", bufs=4) as out_pool,
            tc.tile_pool(name="outr", bufs=2) as outr_pool,
            tc.tile_pool(name="psum", bufs=3, space="PSUM") as psum_pool,
            tc.tile_pool(name="wps", bufs=1, space="PSUM") as wps_pool,
        ):
            stat_t = stat_pool.tile([128, STATW], f16)
            rem_t = rem_pool.tile([128, NPAIR * FREE], f16)

            # a handful of warm matmuls ride the head DMA wait: they pre-pay
            # the PE pstate/clock ramp on garbage data so the first real
            # matmuls run at full clock.
            warm_t = warm_pool.tile([128, 640], f16)
            nc.vector.memset(warm_t[:, :], 0.0)
            wps_t = wps_pool.tile([128, 512], f32, tag="w")
            for _ in range(5):
                nc.tensor.matmul(
                    wps_t[:, :], warm_t[:, 0:128], warm_t[:, 128:640],
                    start=True, stop=True,
                )

            # each SWDGE trigger's descriptors drain on ONE DMA queue, so a
            # piece is split into two 64-row triggers to engage two queues in
            # parallel (halves piece latency; descriptor count is free).
            def load_stat(c0, c1):
                for lo, hi in ((0, 64), (64, 128)):
                    nc.gpsimd.dma_start(
                        out=stat_t[lo:hi, c0 * 128 : c1 * 128],
                        in_=stat_dram[lo:hi, c0 * 128 : c1 * 128],
                    )

            rhs_tiles = [None] * SPC

            def fetch(b, piece, ways=4):
                t0, t1 = piece
                rt = rhs_tiles[b]
                if rt is None:
                    rt = rhs_pool.tile([128, NT * FREE], f16, tag="rhs")
                    rhs_tiles[b] = rt
                splits = (
                    ((0, 32), (32, 64), (64, 96), (96, KC))
                    if ways == 4
                    else ((0, 64), (64, KC))
                )
                for lo, hi in splits:
                    nc.gpsimd.dma_start(
                        out=rt[lo:hi, t0 * FREE : t1 * FREE],
                        in_=im_dram[b, lo:hi, t0:t1],
                    )

            def fetch_rem(p):
                for lo, hi in ((0, 64), (64, REM_KC)):
                    nc.gpsimd.dma_start(
                        out=rem_t[lo:hi, p * FREE : (p + 1) * FREE],
                        in_=rem_dram[p, lo:hi, :],
                    )

            # upfront DMA order: the first tile's deps first, then pieces
            # sized to land just ahead of each tile's compute deadline.
            fetch(0, IN_PIECES_B0[0], ways=2)
            load_stat(0, 5)  # sample 0 class 0 (tile 0)
            fetch(0, IN_PIECES_B0[1], ways=2)
            load_stat(5, 20)  # b0 cls1 + b1 cls0/1 head
            for p in IN_PIECES_B0[2:4]:
                fetch(0, p)
            load_stat(20, NBLK)  # the rest (incl. remainder blocks)
            for p in IN_PIECES_B0[4:]:
                fetch(0, p)
            fetch(1, IN_PIECES[0])
            pending = [
                ("in", 1, IN_PIECES[1]),
                ("in", 1, IN_PIECES[2]),
                ("rem", 0, None),
                ("in", 2, IN_PIECES[0]),
                ("in", 2, IN_PIECES[1]),
                ("in", 2, IN_PIECES[2]),
                ("rem", 1, None),
                ("in", 3, IN_PIECES[0]),
                ("in", 3, IN_PIECES[1]),
                ("in", 3, IN_PIECES[2]),
            ]
            pending.reverse()  # pop() from the front-most piece

            def emit_out(b, p0, p1, out_t):
                w_piece = (p1 - p0) * W
                nc.gpsimd.dma_start(
                    out=out_dram[b, 0:64, p0:p1], in_=out_t[0:64, 0:w_piece]
                )
                nc.gpsimd.dma_start(
                    out=out_dram[b, 64:MT_FULL, p0:p1],
                    in_=out_t[64:MT_FULL, 0:w_piece],
                )

            def do_fetch():
                if pending:
                    kind, a0, a1 = pending.pop()
                    if kind == "in":
                        fetch(a0, a1)
                    else:
                        fetch_rem(a0)

            held = None
            for b in range(SPC):
                rhs_t = rhs_tiles[b]
                pieces = OUT_PIECES_LAST if b == SPC - 1 else OUT_PIECES
                for p0, p1 in pieces:
                    out_t = out_pool.tile([MT_FULL, OPC * W], f16, tag="ob")
                    for t in range(p0, p1):
                        cls = 0 if t == 0 else 1
                        roff = t * FREE
                        ooff = (t - p0) * W

                        psA = psum_pool.tile([MT_FULL, 512], f32, tag="psA")
                        psB = psum_pool.tile([MT_FULL, 256], f32, tag="psB")
                        # chunk-outer: back-to-back matmuls stay in the same
                        # accumulation group/psum bank (switching every matmul
                        # costs ~37ns of pipeline drain each).  512-col chunk
                        # first: the ~118ns LDWEIGHTS hides behind the ~213ns
                        # matmuls (it spills ~11ns behind a 384-col one).
                        for cs, nsz, ps in ((0, 512, psA), (512, 256, psB)):
                            for dxi in range(5):
                                col = ((b * NCLS + cls) * 5 + dxi) * 128
                                nc.tensor.matmul(
                                    ps[:, :],
                                    stat_t[:KC, col : col + MT_FULL],
                                    rhs_t[:KC, roff + cs + dxi : roff + cs + dxi + nsz],
                                    start=(dxi == 0),
                                    stop=(dxi == 4),
                                )
                        nc.vector.tensor_copy(
                            out=out_t[:, ooff : ooff + 512], in_=psA[:, :]
                        )
                        nc.scalar.copy(
                            out=out_t[:, ooff + 512 : ooff + 768], in_=psB[:, :]
                        )

                    # piece finished: stream it out, then keep prefetch ahead.
                    # delay each piece's out-DMA by one piece so its copies are
                    # long done when the ring head reaches it; the final
                    # sample's pieces go out immediately to keep the tail tiny.
                    if held is not None:
                        emit_out(*held)
                        held = None
                    if b == SPC - 1:
                        emit_out(b, p0, p1, out_t)
                    else:
                        held = (b, p0, p1, out_t)
                    do_fetch()
                    do_fetch()

                if b % 2 == 1:
                    # remainder strip for the sample pair (b-1, b)
                    p = b // 2
                    psA = psum_pool.tile([MT_FULL, 512], f32, tag="psA")
                    psB = psum_pool.tile([MT_FULL, 256], f32, tag="psB")
                    for cs, nsz, ps in ((0, 512, psA), (512, 256, psB)):
                        for dxi in range(5):
                            col = (SPC * NSTAT + p * 5 + dxi) * 128
                            off = p * FREE + cs + dxi
                            nc.tensor.matmul(
                                ps[:REM_MT, :],
                                stat_t[:REM_KC, col : col + REM_MT],
                                rem_t[:REM_KC, off : off + nsz],
                                start=(dxi == 0),
                                stop=(dxi == 4),
                            )
                    outr_t = outr_pool.tile([REM_MT, W], f16, tag="or")
                    nc.vector.tensor_copy(out=outr_t[:, 0:512], in_=psA[:REM_MT, :])
                    nc.scalar.copy(out=outr_t[:, 512:768], in_=psB[:REM_MT, :])
                    nc.gpsimd.dma_start(out=outr_dram[p], in_=outr_t[:, :])

            if held is not None:
                emit_out(*held)

    _legalize_waits(nc)
    return nc


def _get_program():
    if "nc" not in _nc_cache:
        _nc_cache["nc"] = _build_program()
    return _nc_cache["nc"]


def _build_stats(flat_col):
    """flat_col [BS, 37] float32 -> (main [BS, NCLS, 5, 128, 128],
    rem [BS//2, 5, 128, 128]) float32."""
    flat_col = np.asarray(flat_col, np.float64)
    nb = flat_col.shape[0]
    geoms = [(0, 0), (ROWS, 1)]  # (tile start row, class id)

    # index arrays per class (independent of sample and dx)
    cls_idx = []
    for a, cls in geoms:
        base = a - 2
        ks, ms, ds, cs, dys = [], [], [], [], []
        ones_m = []
        ones_c = []
        ones_mask = np.zeros((MT_FULL, 5), np.float64)
        for c in range(3):
            for i in range(ROWS):
                m = c * ROWS + i
                h = a + i
                for dy in range(-2, 3):
                    h2 = h + dy
                    if 0 <= h2 < H:
                        ones_mask[m, dy + 2] = 1.0
                        for d in range(3):
                            ks.append(d * WIN + (h2 - base))
                            ms.append(m)
                            ds.append(d)
                            cs.append(c)
                            dys.append(dy + 2)
                ones_m.append(m)
                ones_c.append(c)
        cls_idx.append(
            (
                np.array(ks),
                np.array(ms),
                np.array(ds),
                np.array(cs),
                np.array(dys),
                np.array(ones_m),
                np.array(ones_c),
                ones_mask,
            )
        )

    # remainder index arrays (two samples packed; s = sample within pair)
    r_ks, r_ms, r_ds, r_cs, r_dys, r_ss = [], [], [], [], [], []
    r_ones_m, r_ones_c, r_ones_s = [], [], []
    r_ones_mask = np.zeros((REM_MT, 5), np.float64)
    rbase = REM_A - 2
    for s in range(2):
        for c in range(3):
            for i in range(REM_ROWS):
                m = s * 3 * REM_ROWS + c * REM_ROWS + i
                h = REM_A + i
                for dy in range(-2, 3):
                    h2 = h + dy
                    if 0 <= h2 < H:
                        r_ones_mask[m, dy + 2] = 1.0
                        for d in range(3):
                            r_ks.append(s * 3 * REM_WIN + d * REM_WIN + (h2 - rbase))
                            r_ms.append(m)
                            r_ds.append(d)
                            r_cs.append(c)
                            r_dys.append(dy + 2)
                            r_ss.append(s)
                r_ones_m.append(m)
                r_ones_c.append(c)
                r_ones_s.append(s)
    r_ks = np.array(r_ks)
    r_ms = np.array(r_ms)
    r_ds = np.array(r_ds)
    r_cs = np.array(r_cs)
    r_dys = np.array(r_dys)
    r_ss = np.array(r_ss)
    r_ones_m = np.array(r_ones_m)
    r_ones_c = np.array(r_ones_c)
    r_ones_s = np.array(r_ones_s)

    main = np.zeros((nb, NCLS, 5, 128, 128), np.float32)
    rem = np.zeros((nb // 2, 5, 128, 128), np.float32)
    W3s, sps, k5s = [], [], []
    for b in range(nb):
        W3 = flat_col[b, :9].reshape(3, 3)  # [d, c]
        shift = flat_col[b, 9:12]
        k5 = flat_col[b, 12:37].reshape(5, 5)
        sp = W3.T @ shift
        W3s.append(W3)
        sps.append(sp)
        k5s.append(k5)
        for cls in range(NCLS):
            ks, ms, ds, cs, dys, ones_m, ones_c, ones_mask = cls_idx[cls]
            wvals = W3[ds, cs]
            for dxi in range(5):
                S = np.zeros((128, 128), np.float32)
                S[ks, ms] = (wvals * k5[dys, dxi]).astype(np.float32)
                S[ONES, ones_m] = (sp[ones_c] * (ones_mask @ k5[:, dxi])).astype(
                    np.float32
                )
                main[b, cls, dxi] = S
    for p in range(nb // 2):
        for dxi in range(5):
            S = np.zeros((128, 128), np.float32)
            for s in range(2):
                b = 2 * p + s
                sel = r_ss == s
                S[r_ks[sel], r_ms[sel]] = (
                    W3s[b][r_ds[sel], r_cs[sel]] * k5s[b][r_dys[sel], dxi]
                ).astype(np.float32)
                osel = r_ones_s == s
                S[REM_ONES, r_ones_m[osel]] = (
                    sps[b][r_ones_c[osel]]
                    * (r_ones_mask[r_ones_m[osel]] @ k5s[b][:, dxi])
                ).astype(np.float32)
            rem[p, dxi] = S
    return main, rem


def _prep_inputs(im, flat_col):
    im = np.asarray(im, dtype=np.float32)
    stats, rstats = _build_stats(flat_col)

    im_pad = np.zeros((BS, C, H + 4, FREE), np.float32)
    im_pad[:, :, 2 : 2 + H, 2 : 2 + W] = im

    im_tiled = np.zeros((BS, KC, NT, FREE), np.float16)
    for t in range(NT):
        a = t * ROWS
        im_tiled[:, : 3 * WIN, t] = im_pad[:, :, a : a + WIN, :].reshape(
            BS, 3 * WIN, FREE
        )
        im_tiled[:, ONES, t, 2 : 2 + W] = 1.0

    # remainder strip: rows 758..770 (window of outputs 760..768), two
    # samples stacked in the contraction dim.
    im_rem = np.zeros((BS // 2, REM_KC, FREE), np.float16)
    rwin = im_pad[:, :, REM_A : REM_A + REM_WIN, :].reshape(BS, 3 * REM_WIN, FREE)
    im_rem[:, : 3 * REM_WIN] = rwin[0::2]
    im_rem[:, 3 * REM_WIN : 6 * REM_WIN] = rwin[1::2]
    im_rem[:, REM_ONES, 2 : 2 + W] = 1.0

    in_maps = []
    for ci in range(NCORES):
        sl = slice(ci * SPC, (ci + 1) * SPC)
        psl = slice(ci * NPAIR, (ci + 1) * NPAIR)
        # [SPC, NCLS, 5, 128k, 128m] -> [128k, SPC*NCLS*5*128m], then append
        # the remainder blocks [NPAIR, 5, 128k, 128m]
        st_main = stats[sl].transpose(3, 0, 1, 2, 4).reshape(128, SPC * NSTAT * 128)
        st_rem = rstats[psl].transpose(2, 0, 1, 3).reshape(128, NPAIR * 5 * 128)
        st = np.ascontiguousarray(np.concatenate([st_main, st_rem], axis=1))
        in_maps.append(
            {
                "im_tiled": np.ascontiguousarray(im_tiled[sl]),
                "im_rem": np.ascontiguousarray(im_rem[psl]),
                "stat": st.astype(np.float16),
            }
        )
    return in_maps


def _unpack_output(res):
    out = np.empty((BS, C, H, W), np.float32)
    for ci, r in enumerate(res.results):
        ot = r["out_tiled"]  # [SPC, MT_FULL, NT, W] fp16
        sl = slice(ci * SPC, (ci + 1) * SPC)
        out[sl, :, : NT * ROWS, :] = (
            ot.transpose(0, 2, 1, 3)
            .reshape(SPC, NT, 3, ROWS, W)
            .transpose(0, 2, 1, 3, 4)
            .reshape(SPC, 3, NT * ROWS, W)
        )
        orem = r["out_rem"].reshape(NPAIR, 2, 3, REM_ROWS, W)
        for p in range(NPAIR):
            for s in range(2):
                out[ci * SPC + 2 * p + s, :, REM_A:, :] = orem[p, s]
    return out


def _run(im, flat_col, trace=False, **trace_kwargs):
    nc = _get_program()
    in_maps = _prep_inputs(im, flat_col)
    res = run_bass_kernel_spmd(
        nc, in_maps, list(range(NCORES)), trace=trace, **trace_kwargs
    )
    return _unpack_output(res), res


def kernel(im, flat_col):
    out, _ = _run(im, flat_col)
    return out


# revision 23
# speedup vs baseline: 1.3614x; 1.3614x over previous
"""ColWarp (per-sample color warp + shift + depthwise 5x5 conv) on 8 TRN2 cores.

Decomposition: out[c] = conv5x5(sum_d W[d,c]*(im[d]+shift[d]), k) is linear,
so the 3x3 color warp, the per-channel shift, and the 5-tap column conv all
fold into per-sample banded Toeplitz stationary matrices built on host from
flat_col (32x37).  Each output row-tile is 5 PSUM-accumulated fp16 matmul
pairs (one per kernel column dx, the dx shift expressed as a free-dim offset
into a zero-padded rhs tile).  The shift term rides on a constant ones row
whose stationary coefficient is s'[c] * (sum of dy-valid kernel taps), which
reproduces the zero-padding border behavior exactly.

v2 schedule (PE is the bottleneck at ~131us busy; everything else must hide
behind it):
  - 20 full 38-row h-tiles per sample; the 8-row remainder strip of two
    samples is packed into one 73-contraction stationary (48 psum rows), so
    the remainder costs 2 pass-sets instead of 4.
  - dx-outer matmul order with two 384-wide psum chunks per tile: every
    LDWEIGHTS (~118ns) hides behind a 384-col matmul (~160ns), instead of
    spilling ~8ns behind each 256-col matmul.
  - lean head: first input tile + first 5 stationaries are DMA'd first and
    a short warmup ramp (NWARM matmuls) covers exactly the DMA wait.
  - psum->sbuf copies split across vector AND scalar engines per tile.
  - output pieces stream held-by-one; the final pieces shrink so the drain
    after the last matmul is small.

Data parallel: 4 samples per core, 8 cores, no cross-core communication.
"""

import numpy as np

import concourse.bass as bass
import concourse.mybir as mybir
import concourse.tile as tile
from concourse.bass_utils import run_bass_kernel_spmd

BS, C, H, W = 32, 3, 768, 768
NCORES = 8
SPC = BS // NCORES  # 4 samples per core

ROWS = 38  # output rows per full h-tile
WIN = ROWS + 4  # input window rows per channel (42)
NT = 20  # full h-tiles (rows 0..760); remainder strip handled separately
KC = 3 * WIN + 1  # matmul contraction: 126 image rows + ones row
ONES = 3 * WIN  # partition index of the ones row (126)
FREE = W + 4  # rhs width incl. 2+2 zero side cols (772)
NCLS = 2  # stationary classes: first / interior tile
MT_FULL = 3 * ROWS  # 114
OPC = 7  # max h-tiles per output DMA piece

# remainder strip: rows 760..768, two samples packed per pass-set
REM_ROWS = H - NT * ROWS  # 8
REM_WIN = REM_ROWS + 4  # 12
REM_KC = 2 * 3 * REM_WIN + 1  # 73 (two samples' windows + ones row)
REM_ONES = 2 * 3 * REM_WIN  # 72
REM_MT = 2 * 3 * REM_ROWS  # 48
REM_A = NT * ROWS  # 760
NPAIR = SPC // 2  # 2

NSTAT = NCLS * 5  # main stationaries per sample (10)
NBLK = SPC * NSTAT + NPAIR * 5  # 50 stationary blocks total
STATW = NBLK * 128

# input DMA pieces (tile ranges) per sample: sample 0 starts fine-grained so
# the first matmul's data lands early; the rest use full-width pieces.
IN_PIECES_B0 = [(0, 1), (1, 2), (2, 4), (4, 7), (7, 10), (10, 14), (14, 17), (17, 20)]
IN_PIECES = [(0, 7), (7, 14), (14, 20)]
OUT_PIECES = [(0, 7), (7, 14), (14, 20)]
# last sample's pieces shrink so the end-of-kernel drain is tiny.
OUT_PIECES_LAST = [(0, 7), (7, 14), (14, 17), (17, 19), (19, 20)]

_nc_cache = {}


def _legalize_waits(nc):
    # This walrus build rejects >1 sync wait per instruction; move extra
    # waits onto same-engine NOPs immediately before (sequencers execute
    # waits in program order, so this is equivalent).
    for f in nc.m.functions:
        for blk in f.blocks:
            out = []
            changed = False
            for inst in blk.instructions:
                si = inst.sync_info
                waits = list(si.on_wait) if si is not None and si.on_wait else []
                if len(waits) > 1:
                    changed = True
                    for j, w in enumerate(waits[:-1]):
                        out.append(
                            mybir.InstNoOp(
                                name=f"{inst.name}-wsplit{j}",
                                engine=inst.engine,
                                ins=[],
                                outs=[],
                                sync_info=mybir.SyncInfo(on_wait=[w], on_update=[]),
                            )
                        )
                    inst.sync_info = mybir.SyncInfo(
                        on_wait=[waits[-1]],
                        on_update=list(si.on_update) if si.on_update else [],
                    )
                out.append(inst)
            if changed:
                blk.instructions = out


def _build_program():
    f32 = mybir.dt.float32
    f16 = mybir.dt.float16
    nc = bass.Bass()
    im_dram = nc.declare_dram_parameter(
        "im_tiled", [SPC, KC, NT, FREE], f16, isOutput=False
    )
    rem_dram = nc.declare_dram_parameter(
        "im_rem", [NPAIR, REM_KC, FREE], f16, isOutput=False
    )
    stat_dram = nc.declare_dram_parameter("stat", [128, STATW], f16, isOutput=False)
    out_dram = nc.declare_dram_parameter(
        "out_tiled", [SPC, MT_FULL, NT, W], f16, isOutput=True
    )
    outr_dram = nc.declare_dram_parameter(
        "out_rem", [NPAIR, REM_MT, W], f16, isOutput=True
    )

    with tile.TileContext(nc) as tc:
        with (
            tc.tile_pool(name="stat", bufs=1) as stat_pool,
            tc.tile_pool(name="rem", bufs=1) as rem_pool,
            tc.tile_pool(name="warm", bufs=1) as warm_pool,
            tc.tile_pool(name="rhs", bufs=3) as rhs_pool,
            tc.tile_pool(name="outb", bufs=4) as out_pool,
            tc.tile_pool(name="outr", bufs=2) as outr_pool,
            tc.tile_pool(name="psum", bufs=3, space="PSUM") as psum_pool,
            tc.tile_pool(name="wps", bufs=1, space="PSUM") as wps_pool,
        ):
            stat_t = stat_pool.tile([128, STATW], f16)
            rem_t = rem_pool.tile([128, NPAIR * FREE], f16)

            # a handful of warm matmuls ride the head DMA wait: they pre-pay
            # the PE pstate/clock ramp on garbage data so the first real
            # matmuls run at full clock.
            warm_t = warm_pool.tile([128, 640], f16)
            nc.vector.memset(warm_t[:, :], 0.0)
            wps_t = wps_pool.tile([128, 512], f32, tag="w")
            for _ in range(5):
                nc.tensor.matmul(
                    wps_t[:, :], warm_t[:, 0:128], warm_t[:, 128:640],
                    start=True, stop=True,
                )

            # each SWDGE trigger's descriptors drain on ONE DMA queue, so a
            # piece is split into two 64-row triggers to engage two queues in
            # parallel (halves piece latency; descriptor count is free).
            def load_stat(c0, c1):
                for lo, hi in ((0, 64), (64, 128)):
                    nc.gpsimd.dma_start(
                        out=stat_t[lo:hi, c0 * 128 : c1 * 128],
                        in_=stat_dram[lo:hi, c0 * 128 : c1 * 128],
                    )

            rhs_tiles = [None] * SPC

            def fetch(b, piece):
                t0, t1 = piece
                rt = rhs_tiles[b]
                if rt is None:
                    rt = rhs_pool.tile([128, NT * FREE], f16, tag="rhs")
                    rhs_tiles[b] = rt
                for lo, hi in ((0, 64), (64, KC)):
                    nc.gpsimd.dma_start(
                        out=rt[lo:hi, t0 * FREE : t1 * FREE],
                        in_=im_dram[b, lo:hi, t0:t1],
                    )

            def fetch_rem(p):
                for lo, hi in ((0, 64), (64, REM_KC)):
                    nc.gpsimd.dma_start(
                        out=rem_t[lo:hi, p * FREE : (p + 1) * FREE],
                        in_=rem_dram[p, lo:hi, :],
                    )

            # upfront DMA order: the first tile's deps first, then pieces
            # sized to land just ahead of each tile's compute deadline.
            fetch(0, IN_PIECES_B0[0])
            load_stat(0, 5)  # sample 0 class 0 (tile 0)
            fetch(0, IN_PIECES_B0[1])
            load_stat(5, 10)  # sample 0 class 1 (tiles 1+)
            for p in IN_PIECES_B0[2:4]:
                fetch(0, p)
            load_stat(10, 20)  # sample 1 stationaries
            for p in IN_PIECES_B0[4:]:
                fetch(0, p)
            load_stat(20, NBLK)  # the rest (incl. remainder blocks)
            fetch(1, IN_PIECES[0])
            pending = [
                ("in", 1, IN_PIECES[1]),
                ("in", 1, IN_PIECES[2]),
                ("rem", 0, None),
                ("in", 2, IN_PIECES[0]),
                ("in", 2, IN_PIECES[1]),
                ("in", 2, IN_PIECES[2]),
                ("rem", 1, None),
                ("in", 3, IN_PIECES[0]),
                ("in", 3, IN_PIECES[1]),
                ("in", 3, IN_PIECES[2]),
            ]
            pending.reverse()  # pop() from the front-most piece

            def emit_out(b, p0, p1, out_t):
                w_piece = (p1 - p0) * W
                nc.gpsimd.dma_start(
                    out=out_dram[b, 0:64, p0:p1], in_=out_t[0:64, 0:w_piece]
                )
                nc.gpsimd.dma_start(
                    out=out_dram[b, 64:MT_FULL, p0:p1],
                    in_=out_t[64:MT_FULL, 0:w_piece],
                )

            def do_fetch():
                if pending:
                    kind, a0, a1 = pending.pop()
                    if kind == "in":
                        fetch(a0, a1)
                    else:
                        fetch_rem(a0)

            held = None
            for b in range(SPC):
                rhs_t = rhs_tiles[b]
                pieces = OUT_PIECES_LAST if b == SPC - 1 else OUT_PIECES
                for p0, p1 in pieces:
                    out_t = out_pool.tile([MT_FULL, OPC * W], f16, tag="ob")
                    for t in range(p0, p1):
                        cls = 0 if t == 0 else 1
                        roff = t * FREE
                        ooff = (t - p0) * W

                        psA = psum_pool.tile([MT_FULL, 512], f32, tag="psA")
                        psB = psum_pool.tile([MT_FULL, 256], f32, tag="psB")
                        # chunk-outer: back-to-back matmuls stay in the same
                        # accumulation group/psum bank (switching every matmul
                        # costs ~37ns of pipeline drain each).  512-col chunk
                        # first: the ~118ns LDWEIGHTS hides behind the ~213ns
                        # matmuls (it spills ~11ns behind a 384-col one).
                        for cs, nsz, ps in ((0, 512, psA), (512, 256, psB)):
                            for dxi in range(5):
                                col = ((b * NCLS + cls) * 5 + dxi) * 128
                                nc.tensor.matmul(
                                    ps[:, :],
                                    stat_t[:KC, col : col + MT_FULL],
                                    rhs_t[:KC, roff + cs + dxi : roff + cs + dxi + nsz],
                                    start=(dxi == 0),
                                    stop=(dxi == 4),
                                )
                        nc.vector.tensor_copy(
                            out=out_t[:, ooff : ooff + 512], in_=psA[:, :]
                        )
                        nc.scalar.copy(
                            out=out_t[:, ooff + 512 : ooff + 768], in_=psB[:, :]
                        )

                    # piece finished: stream it out, then keep prefetch ahead.
                    # delay each piece's out-DMA by one piece so its copies are
                    # long done when the ring head reaches it; the final
                    # sample's pieces go out immediately to keep the tail tiny.
                    if held is not None:
                        emit_out(*held)
                        held = None
                    if b == SPC - 1:
                        emit_out(b, p0, p1, out_t)
                    else:
                        held = (b, p0, p1, out_t)
                    do_fetch()
                    do_fetch()

                if b % 2 == 1:
                    # remainder strip for the sample pair (b-1, b)
                    p = b // 2
                    psA = psum_pool.tile([MT_FULL, 512], f32, tag="psA")
                    psB = psum_pool.tile([MT_FULL, 256], f32, tag="psB")
                    for cs, nsz, ps in ((0, 512, psA), (512, 256, psB)):
                        for dxi in range(5):
                            col = (SPC * NSTAT + p * 5 + dxi) * 128
                            off = p * FREE + cs + dxi
                            nc.tensor.matmul(
                                ps[:REM_MT, :],
                                stat_t[:REM_KC, col : col + REM_MT],
                                rem_t[:REM_KC, off : off + nsz],
                                start=(dxi == 0),
                                stop=(dxi == 4),
                            )
                    outr_t = outr_pool.tile([REM_MT, W], f16, tag="or")
                    nc.vector.tensor_copy(out=outr_t[:, 0:512], in_=psA[:REM_MT, :])
                    nc.scalar.copy(out=outr_t[:, 512:768], in_=psB[:REM_MT, :])
                    nc.gpsimd.dma_start(out=outr_dram[p], in_=outr_t[:, :])

            if held is not None:
                emit_out(*held)

    _legalize_waits(nc)
    return nc


def _get_program():
    if "nc" not in _nc_cache:
        _nc_cache["nc"] = _build_program()
    return _nc_cache["nc"]


def _build_stats(flat_col):
    """flat_col [BS, 37] float32 -> (main [BS, NCLS, 5, 128, 128],
    rem [BS//2, 5, 128, 128]) float32."""
    flat_col = np.asarray(flat_col, np.float64)
    nb = flat_col.shape[0]
    geoms = [(0, 0), (ROWS, 1)]  # (tile start row, class id)

    # index arrays per class (independent of sample and dx)
    cls_idx = []
    for a, cls in geoms:
        base = a - 2
        ks, ms, ds, cs, dys = [], [], [], [], []
        ones_m = []
        ones_c = []
        ones_mask = np.zeros((MT_FULL, 5), np.float64)
        for c in range(3):
            for i in range(ROWS):
                m = c * ROWS + i
                h = a + i
                for dy in range(-2, 3):
                    h2 = h + dy
                    if 0 <= h2 < H:
                        ones_mask[m, dy + 2] = 1.0
                        for d in range(3):
                            ks.append(d * WIN + (h2 - base))
                            ms.append(m)
                            ds.append(d)
                            cs.append(c)
                            dys.append(dy + 2)
                ones_m.append(m)
                ones_c.append(c)
        cls_idx.append(
            (
                np.array(ks),
                np.array(ms),
                np.array(ds),
                np.array(cs),
                np.array(dys),
                np.array(ones_m),
                np.array(ones_c),
                ones_mask,
            )
        )

    # remainder index arrays (two samples packed; s = sample within pair)
    r_ks, r_ms, r_ds, r_cs, r_dys, r_ss = [], [], [], [], [], []
    r_ones_m, r_ones_c, r_ones_s = [], [], []
    r_ones_mask = np.zeros((REM_MT, 5), np.float64)
    rbase = REM_A - 2
    for s in range(2):
        for c in range(3):
            for i in range(REM_ROWS):
                m = s * 3 * REM_ROWS + c * REM_ROWS + i
                h = REM_A + i
                for dy in range(-2, 3):
                    h2 = h + dy
                    if 0 <= h2 < H:
                        r_ones_mask[m, dy + 2] = 1.0
                        for d in range(3):
                            r_ks.append(s * 3 * REM_WIN + d * REM_WIN + (h2 - rbase))
                            r_ms.append(m)
                            r_ds.append(d)
                            r_cs.append(c)
                            r_dys.append(dy + 2)
                            r_ss.append(s)
                r_ones_m.append(m)
                r_ones_c.append(c)
                r_ones_s.append(s)
    r_ks = np.array(r_ks)
    r_ms = np.array(r_ms)
    r_ds = np.array(r_ds)
    r_cs = np.array(r_cs)
    r_dys = np.array(r_dys)
    r_ss = np.array(r_ss)
    r_ones_m = np.array(r_ones_m)
    r_ones_c = np.array(r_ones_c)
    r_ones_s = np.array(r_ones_s)

    main = np.zeros((nb, NCLS, 5, 128, 128), np.float32)
    rem = np.zeros((nb // 2, 5, 128, 128), np.float32)
    W3s, sps, k5s = [], [], []
    for b in range(nb):
        W3 = flat_col[b, :9].reshape(3, 3)  # [d, c]
        shift = flat_col[b, 9:12]
        k5 = flat_col[b, 12:37].reshape(5, 5)
        sp = W3.T @ shift
        W3s.append(W3)
        sps.append(sp)
        k5s.append(k5)
        for cls in range(NCLS):
            ks, ms, ds, cs, dys, ones_m, ones_c, ones_mask = cls_idx[cls]
            wvals = W3[ds, cs]
            for dxi in range(5):
                S = np.zeros((128, 128), np.float32)
                S[ks, ms] = (wvals * k5[dys, dxi]).astype(np.float32)
                S[ONES, ones_m] = (sp[ones_c] * (ones_mask @ k5[:, dxi])).astype(
                    np.float32
                )
                main[b, cls, dxi] = S
    for p in range(nb // 2):
        for dxi in range(5):
            S = np.zeros((128, 128), np.float32)
            for s in range(2):
                b = 2 * p + s
                sel = r_ss == s
                S[r_ks[sel], r_ms[sel]] = (
                    W3s[b][r_ds[sel], r_cs[sel]] * k5s[b][r_dys[sel], dxi]
                ).astype(np.float32)
                osel = r_ones_s == s
                S[REM_ONES, r_ones_m[osel]] = (
                    sps[b][r_ones_c[osel]]
                    * (r_ones_mask[r_ones_m[osel]] @ k5s[b][:, dxi])
                ).astype(np.float32)
            rem[p, dxi] = S
    return main, rem


def _prep_inputs(im, flat_col):
    im = np.asarray(im, dtype=np.float32)
    stats, rstats = _build_stats(flat_col)

    im_pad = np.zeros((BS, C, H + 4, FREE), np.float32)
    im_pad[:, :, 2 : 2 + H, 2 : 2 + W] = im

    im_tiled = np.zeros((BS, KC, NT, FREE), np.float16)
    for t in range(NT):
        a = t * ROWS
        im_tiled[:, : 3 * WIN, t] = im_pad[:, :, a : a + WIN, :].reshape(
            BS, 3 * WIN, FREE
        )
        im_tiled[:, ONES, t, 2 : 2 + W] = 1.0

    # remainder strip: rows 758..770 (window of outputs 760..768), two
    # samples stacked in the contraction dim.
    im_rem = np.zeros((BS // 2, REM_KC, FREE), np.float16)
    rwin = im_pad[:, :, REM_A : REM_A + REM_WIN, :].reshape(BS, 3 * REM_WIN, FREE)
    im_rem[:, : 3 * REM_WIN] = rwin[0::2]
    im_rem[:, 3 * REM_WIN : 6 * REM_WIN] = rwin[1::2]
    im_rem[:, REM_ONES, 2 : 2 + W] = 1.0

    in_maps = []
    for ci in range(NCORES):
        sl = slice(ci * SPC, (ci + 1) * SPC)
        psl = slice(ci * NPAIR, (ci + 1) * NPAIR)
        # [SPC, NCLS, 5, 128k, 128m] -> [128k, SPC*NCLS*5*128m], then append
        # the remainder blocks [NPAIR, 5, 128k, 128m]
        st_main = stats[sl].transpose(3, 0, 1, 2, 4).reshape(128, SPC * NSTAT * 128)
        st_rem = rstats[psl].transpose(2, 0, 1, 3).reshape(128, NPAIR * 5 * 128)
        st = np.ascontiguousarray(np.concatenate([st_main, st_rem], axis=1))
        in_maps.append(
            {
                "im_tiled": np.ascontiguousarray(im_tiled[sl]),
                "im_rem": np.ascontiguousarray(im_rem[psl]),
                "stat": st.astype(np.float16),
            }
        )
    return in_maps


def _unpack_output(res):
    out = np.empty((BS, C, H, W), np.float32)
    for ci, r in enumerate(res.results):
        ot = r["out_tiled"]  # [SPC, MT_FULL, NT, W] fp16
        sl = slice(ci * SPC, (ci + 1) * SPC)
        out[sl, :, : NT * ROWS, :] = (
            ot.transpose(0, 2, 1, 3)
            .reshape(SPC, NT, 3, ROWS, W)
            .transpose(0, 2, 1, 3, 4)
            .reshape(SPC, 3, NT * ROWS, W)
        )
        orem = r["out_rem"].reshape(NPAIR, 2, 3, REM_ROWS, W)
        for p in range(NPAIR):
            for s in range(2):
                out[ci * SPC + 2 * p + s, :, REM_A:, :] = orem[p, s]
    return out


def _run(im, flat_col, trace=False, **trace_kwargs):
    nc = _get_program()
    in_maps = _prep_inputs(im, flat_col)
    res = run_bass_kernel_spmd(
        nc, in_maps, list(range(NCORES)), trace=trace, **trace_kwargs
    )
    return _unpack_output(res), res


def kernel(im, flat_col):
    out, _ = _run(im, flat_col)
    return out


# revision 24
# speedup vs baseline: 1.3646x; 1.0023x over previous
"""ColWarp (per-sample color warp + shift + depthwise 5x5 conv) on 8 TRN2 cores.

Decomposition: out[c] = conv5x5(sum_d W[d,c]*(im[d]+shift[d]), k) is linear,
so the 3x3 color warp, the per-channel shift, and the 5-tap column conv all
fold into per-sample banded Toeplitz stationary matrices built on host from
flat_col (32x37).  Each output row-tile is 5 PSUM-accumulated fp16 matmul
pairs (one per kernel column dx, the dx shift expressed as a free-dim offset
into a zero-padded rhs tile).  The shift term rides on a constant ones row
whose stationary coefficient is s'[c] * (sum of dy-valid kernel taps), which
reproduces the zero-padding border behavior exactly.

v2 schedule (PE is the bottleneck at ~131us busy; everything else must hide
behind it):
  - 20 full 38-row h-tiles per sample; the 8-row remainder strip of two
    samples is packed into one 73-contraction stationary (48 psum rows), so
    the remainder costs 2 pass-sets instead of 4.
  - dx-outer matmul order with two 384-wide psum chunks per tile: every
    LDWEIGHTS (~118ns) hides behind a 384-col matmul (~160ns), instead of
    spilling ~8ns behind each 256-col matmul.
  - lean head: first input tile + first 5 stationaries are DMA'd first and
    a short warmup ramp (NWARM matmuls) covers exactly the DMA wait.
  - psum->sbuf copies split across vector AND scalar engines per tile.
  - output pieces stream held-by-one; the final pieces shrink so the drain
    after the last matmul is small.

Data parallel: 4 samples per core, 8 cores, no cross-core communication.
"""

import os

import numpy as np

# Ask the runtime for a fresh core state: a prior run that left the clock
# governor in a degraded state otherwise taxes this run ~20%.
os.environ.setdefault("NEURON_RT_RESET_CORES", "1")

import concourse.bass as bass
import concourse.mybir as mybir
import concourse.tile as tile
from concourse.bass_utils import run_bass_kernel_spmd

BS, C, H, W = 32, 3, 768, 768
NCORES = 8
SPC = BS // NCORES  # 4 samples per core

ROWS = 38  # output rows per full h-tile
WIN = ROWS + 4  # input window rows per channel (42)
NT = 20  # full h-tiles (rows 0..760); remainder strip handled separately
KC = 3 * WIN + 1  # matmul contraction: 126 image rows + ones row
ONES = 3 * WIN  # partition index of the ones row (126)
FREE = W + 4  # rhs width incl. 2+2 zero side cols (772)
NCLS = 2  # stationary classes: first / interior tile
MT_FULL = 3 * ROWS  # 114
OPC = 7  # max h-tiles per output DMA piece

# remainder strip: rows 760..768, two samples packed per pass-set
REM_ROWS = H - NT * ROWS  # 8
REM_WIN = REM_ROWS + 4  # 12
REM_KC = 2 * 3 * REM_WIN + 1  # 73 (two samples' windows + ones row)
REM_ONES = 2 * 3 * REM_WIN  # 72
REM_MT = 2 * 3 * REM_ROWS  # 48
REM_A = NT * ROWS  # 760
NPAIR = SPC // 2  # 2

NSTAT = NCLS * 5  # main stationaries per sample (10)
NBLK = SPC * NSTAT + NPAIR * 5  # 50 stationary blocks total
STATW = NBLK * 128

# input DMA pieces (tile ranges) per sample: sample 0 starts fine-grained so
# the first matmul's data lands early; the rest use full-width pieces.
IN_PIECES_B0 = [(0, 1), (1, 2), (2, 4), (4, 7), (7, 10), (10, 14), (14, 17), (17, 20)]
IN_PIECES = [(0, 7), (7, 14), (14, 20)]
OUT_PIECES = [(0, 7), (7, 14), (14, 20)]
# last sample's pieces shrink so the end-of-kernel drain is tiny.
OUT_PIECES_LAST = [(0, 7), (7, 14), (14, 17), (17, 19), (19, 20)]

_nc_cache = {}


def _legalize_waits(nc):
    # This walrus build rejects >1 sync wait per instruction; move extra
    # waits onto same-engine NOPs immediately before (sequencers execute
    # waits in program order, so this is equivalent).
    for f in nc.m.functions:
        for blk in f.blocks:
            out = []
            changed = False
            for inst in blk.instructions:
                si = inst.sync_info
                waits = list(si.on_wait) if si is not None and si.on_wait else []
                if len(waits) > 1:
                    changed = True
                    for j, w in enumerate(waits[:-1]):
                        out.append(
                            mybir.InstNoOp(
                                name=f"{inst.name}-wsplit{j}",
                                engine=inst.engine,
                                ins=[],
                                outs=[],
                                sync_info=mybir.SyncInfo(on_wait=[w], on_update=[]),
                            )
                        )
                    inst.sync_info = mybir.SyncInfo(
                        on_wait=[waits[-1]],
                        on_update=list(si.on_update) if si.on_update else [],
                    )
                out.append(inst)
            if changed:
                blk.instructions = out


def _build_program():
    f32 = mybir.dt.float32
    f16 = mybir.dt.float16
    nc = bass.Bass()
    im_dram = nc.declare_dram_parameter(
        "im_tiled", [SPC, KC, NT, FREE], f16, isOutput=False
    )
    rem_dram = nc.declare_dram_parameter(
        "im_rem", [NPAIR, REM_KC, FREE], f16, isOutput=False
    )
    stat_dram = nc.declare_dram_parameter("stat", [128, STATW], f16, isOutput=False)
    out_dram = nc.declare_dram_parameter(
        "out_tiled", [SPC, MT_FULL, NT, W], f16, isOutput=True
    )
    outr_dram = nc.declare_dram_parameter(
        "out_rem", [NPAIR, REM_MT, W], f16, isOutput=True
    )

    with tile.TileContext(nc) as tc:
        with (
            tc.tile_pool(name="stat", bufs=1) as stat_pool,
            tc.tile_pool(name="rem", bufs=1) as rem_pool,
            tc.tile_pool(name="warm", bufs=1) as warm_pool,
            tc.tile_pool(name="rhs", bufs=3) as rhs_pool,
            tc.tile_pool(name="outb", bufs=4) as out_pool,
            tc.tile_pool(name="outr", bufs=2) as outr_pool,
            tc.tile_pool(name="psum", bufs=3, space="PSUM") as psum_pool,
            tc.tile_pool(name="wps", bufs=1, space="PSUM") as wps_pool,
        ):
            stat_t = stat_pool.tile([128, STATW], f16)
            rem_t = rem_pool.tile([128, NPAIR * FREE], f16)

            # a handful of warm matmuls ride the head DMA wait: they pre-pay
            # the PE pstate/clock ramp on garbage data so the first real
            # matmuls run at full clock.
            warm_t = warm_pool.tile([128, 640], f16)
            nc.vector.memset(warm_t[:, :], 0.0)
            wps_t = wps_pool.tile([128, 512], f32, tag="w")
            for _ in range(5):
                nc.tensor.matmul(
                    wps_t[:, :], warm_t[:, 0:128], warm_t[:, 128:640],
                    start=True, stop=True,
                )

            # each SWDGE trigger's descriptors drain on ONE DMA queue, so a
            # piece is split into two 64-row triggers to engage two queues in
            # parallel (halves piece latency; descriptor count is free).
            def load_stat(c0, c1):
                for lo, hi in ((0, 64), (64, 128)):
                    nc.gpsimd.dma_start(
                        out=stat_t[lo:hi, c0 * 128 : c1 * 128],
                        in_=stat_dram[lo:hi, c0 * 128 : c1 * 128],
                    )

            rhs_tiles = [None] * SPC

            def fetch(b, piece):
                t0, t1 = piece
                rt = rhs_tiles[b]
                if rt is None:
                    rt = rhs_pool.tile([128, NT * FREE], f16, tag="rhs")
                    rhs_tiles[b] = rt
                for lo, hi in ((0, 64), (64, KC)):
                    nc.gpsimd.dma_start(
                        out=rt[lo:hi, t0 * FREE : t1 * FREE],
                        in_=im_dram[b, lo:hi, t0:t1],
                    )

            def fetch_rem(p):
                for lo, hi in ((0, 64), (64, REM_KC)):
                    nc.gpsimd.dma_start(
                        out=rem_t[lo:hi, p * FREE : (p + 1) * FREE],
                        in_=rem_dram[p, lo:hi, :],
                    )

            # upfront DMA order: the first tile's deps first, then pieces
            # sized to land just ahead of each tile's compute deadline.
            fetch(0, IN_PIECES_B0[0])
            load_stat(0, 5)  # sample 0 class 0 (tile 0)
            fetch(0, IN_PIECES_B0[1])
            load_stat(5, 10)  # sample 0 class 1 (tiles 1+)
            for p in IN_PIECES_B0[2:4]:
                fetch(0, p)
            load_stat(10, 20)  # sample 1 stationaries
            for p in IN_PIECES_B0[4:]:
                fetch(0, p)
            load_stat(20, NBLK)  # the rest (incl. remainder blocks)
            fetch(1, IN_PIECES[0])
            pending = [
                ("in", 1, IN_PIECES[1]),
                ("in", 1, IN_PIECES[2]),
                ("rem", 0, None),
                ("in", 2, IN_PIECES[0]),
                ("in", 2, IN_PIECES[1]),
                ("in", 2, IN_PIECES[2]),
                ("rem", 1, None),
                ("in", 3, IN_PIECES[0]),
                ("in", 3, IN_PIECES[1]),
                ("in", 3, IN_PIECES[2]),
            ]
            pending.reverse()  # pop() from the front-most piece

            def emit_out(b, p0, p1, out_t):
                w_piece = (p1 - p0) * W
                nc.gpsimd.dma_start(
                    out=out_dram[b, 0:64, p0:p1], in_=out_t[0:64, 0:w_piece]
                )
                nc.gpsimd.dma_start(
                    out=out_dram[b, 64:MT_FULL, p0:p1],
                    in_=out_t[64:MT_FULL, 0:w_piece],
                )

            def do_fetch():
                if pending:
                    kind, a0, a1 = pending.pop()
                    if kind == "in":
                        fetch(a0, a1)
                    else:
                        fetch_rem(a0)

            held = None
            for b in range(SPC):
                rhs_t = rhs_tiles[b]
                pieces = OUT_PIECES_LAST if b == SPC - 1 else OUT_PIECES
                for p0, p1 in pieces:
                    out_t = out_pool.tile([MT_FULL, OPC * W], f16, tag="ob")
                    for t in range(p0, p1):
                        cls = 0 if t == 0 else 1
                        roff = t * FREE
                        ooff = (t - p0) * W

                        psA = psum_pool.tile([MT_FULL, 512], f32, tag="psA")
                        psB = psum_pool.tile([MT_FULL, 256], f32, tag="psB")
                        # chunk-outer: back-to-back matmuls stay in the same
                        # accumulation group/psum bank (switching every matmul
                        # costs ~37ns of pipeline drain each).  512-col chunk
                        # first: the ~118ns LDWEIGHTS hides behind the ~213ns
                        # matmuls (it spills ~11ns behind a 384-col one).
                        for cs, nsz, ps in ((0, 512, psA), (512, 256, psB)):
                            for dxi in range(5):
                                col = ((b * NCLS + cls) * 5 + dxi) * 128
                                nc.tensor.matmul(
                                    ps[:, :],
                                    stat_t[:KC, col : col + MT_FULL],
                                    rhs_t[:KC, roff + cs + dxi : roff + cs + dxi + nsz],
                                    start=(dxi == 0),
                                    stop=(dxi == 4),
                                )
                        nc.vector.tensor_copy(
                            out=out_t[:, ooff : ooff + 512], in_=psA[:, :]
                        )
                        nc.scalar.copy(
                            out=out_t[:, ooff + 512 : ooff + 768], in_=psB[:, :]
                        )

                    # piece finished: stream it out, then keep prefetch ahead.
                    # delay each piece's out-DMA by one piece so its copies are
                    # long done when the ring head reaches it; the final
                    # sample's pieces go out immediately to keep the tail tiny.
                    if held is not None:
                        emit_out(*held)
                        held = None
                    if b == SPC - 1:
                        emit_out(b, p0, p1, out_t)
                    else:
                        held = (b, p0, p1, out_t)
                    do_fetch()
                    do_fetch()

                if b % 2 == 1:
                    # remainder strip for the sample pair (b-1, b)
                    p = b // 2
                    psA = psum_pool.tile([MT_FULL, 512], f32, tag="psA")
                    psB = psum_pool.tile([MT_FULL, 256], f32, tag="psB")
                    for cs, nsz, ps in ((0, 512, psA), (512, 256, psB)):
                        for dxi in range(5):
                            col = (SPC * NSTAT + p * 5 + dxi) * 128
                            off = p * FREE + cs + dxi
                            nc.tensor.matmul(
                                ps[:REM_MT, :],
                                stat_t[:REM_KC, col : col + REM_MT],
                                rem_t[:REM_KC, off : off + nsz],
                                start=(dxi == 0),
                                stop=(dxi == 4),
                            )
                    outr_t = outr_pool.tile([REM_MT, W], f16, tag="or")
                    nc.vector.tensor_copy(out=outr_t[:, 0:512], in_=psA[:REM_MT, :])
                    nc.scalar.copy(out=outr_t[:, 512:768], in_=psB[:REM_MT, :])
                    nc.gpsimd.dma_start(out=outr_dram[p], in_=outr_t[:, :])

            if held is not None:
                emit_out(*held)

    _legalize_waits(nc)
    return nc


def _get_program():
    if "nc" not in _nc_cache:
        _nc_cache["nc"] = _build_program()
    return _nc_cache["nc"]


def _build_stats(flat_col):
    """flat_col [BS, 37] float32 -> (main [BS, NCLS, 5, 128, 128],
    rem [BS//2, 5, 128, 128]) float32."""
    flat_col = np.asarray(flat_col, np.float64)
    nb = flat_col.shape[0]
    geoms = [(0, 0), (ROWS, 1)]  # (tile start row, class id)

    # index arrays per class (independent of sample and dx)
    cls_idx = []
    for a, cls in geoms:
        base = a - 2
        ks, ms, ds, cs, dys = [], [], [], [], []
        ones_m = []
        ones_c = []
        ones_mask = np.zeros((MT_FULL, 5), np.float64)
        for c in range(3):
            for i in range(ROWS):
                m = c * ROWS + i
                h = a + i
                for dy in range(-2, 3):
                    h2 = h + dy
                    if 0 <= h2 < H:
                        ones_mask[m, dy + 2] = 1.0
                        for d in range(3):
                            ks.append(d * WIN + (h2 - base))
                            ms.append(m)
                            ds.append(d)
                            cs.append(c)
                            dys.append(dy + 2)
                ones_m.append(m)
                ones_c.append(c)
        cls_idx.append(
            (
                np.array(ks),
                np.array(ms),
                np.array(ds),
                np.array(cs),
                np.array(dys),
                np.array(ones_m),
                np.array(ones_c),
                ones_mask,
            )
        )

    # remainder index arrays (two samples packed; s = sample within pair)
    r_ks, r_ms, r_ds, r_cs, r_dys, r_ss = [], [], [], [], [], []
    r_ones_m, r_ones_c, r_ones_s = [], [], []
    r_ones_mask = np.zeros((REM_MT, 5), np.float64)
    rbase = REM_A - 2
    for s in range(2):
        for c in range(3):
            for i in range(REM_ROWS):
                m = s * 3 * REM_ROWS + c * REM_ROWS + i
                h = REM_A + i
                for dy in range(-2, 3):
                    h2 = h + dy
                    if 0 <= h2 < H:
                        r_ones_mask[m, dy + 2] = 1.0
                        for d in range(3):
                            r_ks.append(s * 3 * REM_WIN + d * REM_WIN + (h2 - rbase))
                            r_ms.append(m)
                            r_ds.append(d)
                            r_cs.append(c)
                            r_dys.append(dy + 2)
                            r_ss.append(s)
                r_ones_m.append(m)
                r_ones_c.append(c)
                r_ones_s.append(s)
    r_ks = np.array(r_ks)
    r_ms = np.array(r_ms)
    r_ds = np.array(r_ds)
    r_cs = np.array(r_cs)
    r_dys = np.array(r_dys)
    r_ss = np.array(r_ss)
    r_ones_m = np.array(r_ones_m)
    r_ones_c = np.array(r_ones_c)
    r_ones_s = np.array(r_ones_s)

    main = np.zeros((nb, NCLS, 5, 128, 128), np.float32)
    rem = np.zeros((nb // 2, 5, 128, 128), np.float32)
    W3s, sps, k5s = [], [], []
    for b in range(nb):
        W3 = flat_col[b, :9].reshape(3, 3)  # [d, c]
        shift = flat_col[b, 9:12]
        k5 = flat_col[b, 12:37].reshape(5, 5)
        sp = W3.T @ shift
        W3s.append(W3)
        sps.append(sp)
        k5s.append(k5)
        for cls in range(NCLS):
            ks, ms, ds, cs, dys, ones_m, ones_c, ones_mask = cls_idx[cls]
            wvals = W3[ds, cs]
            for dxi in range(5):
                S = np.zeros((128, 128), np.float32)
                S[ks, ms] = (wvals * k5[dys, dxi]).astype(np.float32)
                S[ONES, ones_m] = (sp[ones_c] * (ones_mask @ k5[:, dxi])).astype(
                    np.float32
                )
                main[b, cls, dxi] = S
    for p in range(nb // 2):
        for dxi in range(5):
            S = np.zeros((128, 128), np.float32)
            for s in range(2):
                b = 2 * p + s
                sel = r_ss == s
                S[r_ks[sel], r_ms[sel]] = (
                    W3s[b][r_ds[sel], r_cs[sel]] * k5s[b][r_dys[sel], dxi]
                ).astype(np.float32)
                osel = r_ones_s == s
                S[REM_ONES, r_ones_m[osel]] = (
                    sps[b][r_ones_c[osel]]
                    * (r_ones_mask[r_ones_m[osel]] @ k5s[b][:, dxi])
                ).astype(np.float32)
            rem[p, dxi] = S
    return main, rem


def _prep_inputs(im, flat_col):
    im = np.asarray(im, dtype=np.float32)
    stats, rstats = _build_stats(flat_col)

    im_pad = np.zeros((BS, C, H + 4, FREE), np.float32)
    im_pad[:, :, 2 : 2 + H, 2 : 2 + W] = im

    im_tiled = np.zeros((BS, KC, NT, FREE), np.float16)
    for t in range(NT):
        a = t * ROWS
        im_tiled[:, : 3 * WIN, t] = im_pad[:, :, a : a + WIN, :].reshape(
            BS, 3 * WIN, FREE
        )
        im_tiled[:, ONES, t, 2 : 2 + W] = 1.0

    # remainder strip: rows 758..770 (window of outputs 760..768), two
    # samples stacked in the contraction dim.
    im_rem = np.zeros((BS // 2, REM_KC, FREE), np.float16)
    rwin = im_pad[:, :, REM_A : REM_A + REM_WIN, :].reshape(BS, 3 * REM_WIN, FREE)
    im_rem[:, : 3 * REM_WIN] = rwin[0::2]
    im_rem[:, 3 * REM_WIN : 6 * REM_WIN] = rwin[1::2]
    im_rem[:, REM_ONES, 2 : 2 + W] = 1.0

    in_maps = []
    for ci in range(NCORES):
        sl = slice(ci * SPC, (ci + 1) * SPC)
        psl = slice(ci * NPAIR, (ci + 1) * NPAIR)
        # [SPC, NCLS, 5, 128k, 128m] -> [128k, SPC*NCLS*5*128m], then append
        # the remainder blocks [NPAIR, 5, 128k, 128m]
        st_main = stats[sl].transpose(3, 0, 1, 2, 4).reshape(128, SPC * NSTAT * 128)
        st_rem = rstats[psl].transpose(2, 0, 1, 3).reshape(128, NPAIR * 5 * 128)
        st = np.ascontiguousarray(np.concatenate([st_main, st_rem], axis=1))
        in_maps.append(
            {
                "im_tiled": np.ascontiguousarray(im_tiled[sl]),
                "im_rem": np.ascontiguousarray(im_rem[psl]),
                "stat": st.astype(np.float16),
            }
        )
    return in_maps


def _unpack_output(res):
    out = np.empty((BS, C, H, W), np.float32)
    for ci, r in enumerate(res.results):
        ot = r["out_tiled"]  # [SPC, MT_FULL, NT, W] fp16
        sl = slice(ci * SPC, (ci + 1) * SPC)
        out[sl, :, : NT * ROWS, :] = (
            ot.transpose(0, 2, 1, 3)
            .reshape(SPC, NT, 3, ROWS, W)
            .transpose(0, 2, 1, 3, 4)
            .reshape(SPC, 3, NT * ROWS, W)
        )
        orem = r["out_rem"].reshape(NPAIR, 2, 3, REM_ROWS, W)
        for p in range(NPAIR):
            for s in range(2):
                out[ci * SPC + 2 * p + s, :, REM_A:, :] = orem[p, s]
    return out


def _run(im, flat_col, trace=False, **trace_kwargs):
    nc = _get_program()
    in_maps = _prep_inputs(im, flat_col)
    res = run_bass_kernel_spmd(
        nc, in_maps, list(range(NCORES)), trace=trace, **trace_kwargs
    )
    return _unpack_output(res), res


def kernel(im, flat_col):
    out, _ = _run(im, flat_col)
    return out
